# revision 1
# baseline (speedup 1.0000x reference)
"""Trainium2 Bass kernel for a 2-layer SimpleRNN classifier.

Model (per reference):
  x = emb[tokens]                               # [B,T,E]
  seq1 = SimpleRNN_relu(x;  W1x, W1h, b1)       # [B,T,H1], return_sequences
  h    = SimpleRNN_relu(seq1; W2x, W2h, b2)[-1] # [B,H2], last step
  h = relu(h@Wd1+bd1); h = relu(h@Wd2+bd2); out = sigmoid(h@Wc+bc)  # [B,1]

Sharding: data-parallel over batch, 8 rows per core on 8 NeuronCores.
All activations are kept *transposed* on-chip (features on partitions,
(time, batch) on the free dim) so:
  - the recurrent state needs no per-step transpose,
  - weights are the PE stationary operand (fp16 -> fast weight load),
  - biases are per-partition vectors fused into ScalarE activations.

Compute dtype: fp16 operands with fp32 PSUM accumulation and fp32
xw (input-projection) buffers.
"""

import numpy as np

import concourse.bass as bass
import concourse.mybir as mybir
import concourse.tile as tile
from concourse.vector_clock import ScopedClock, VectorClock
from concourse.bass_utils import run_bass_kernel_spmd

# ---------------------------------------------------------------------------
# Problem constants (hardcoded per the task contract).
B, T, V, E = 64, 512, 50000, 300
H1, H2, D1, D2, C = 256, 512, 128, 64, 1
N_CORES = 8
BPC = B // N_CORES          # batch rows per core = 8
NT = T * BPC                # columns of the transposed activation = 4096
EP = 384                    # E padded to 3 partition chunks
KE, K1, K2 = EP // 128, H1 // 128, H2 // 128   # 3, 2, 4
BLK = 32                    # time steps per pipeline block
NBLK = T // BLK             # 8
NCOL_BLK = BLK * BPC        # 512 activation columns per block
GATH = NT // 128            # 32 gather tiles of 128 tokens

F16 = mybir.dt.float16
F32 = mybir.dt.float32
I32 = mybir.dt.int32
AF = mybir.ActivationFunctionType


MAX_WAITS = 1  # walrus in this container rejects more sem waits per inst


def _split_excess_waits(nc, max_waits=MAX_WAITS):
    """The container's walrus codegen rejects instructions carrying more than
    a couple of sem waits ("Too many sync wait commands"). Tile freely attaches
    many. Post-process the scheduled BIR: move excess waits onto injected NoOps
    placed immediately before the instruction on the same engine (engines
    process waits in instruction order, so semantics are preserved)."""
    ctr = 0
    for f in nc.m.functions:
        for b in f.blocks:
            new_insts = []
            changed = False
            for inst in b.instructions:
                s = inst.sync_info
                if s is not None and s.on_wait and len(s.on_wait) > max_waits:
                    w = list(s.on_wait)
                    n_extra = len(w) - max_waits
                    for i in range(0, n_extra, max_waits):
                        chunk = w[i : min(i + max_waits, n_extra)]
                        nop = mybir.InstNoOp(
                            name=f"bass_waitsplit_{ctr}",
                            engine=inst.engine,
                            ins=[],
                            outs=[],
                            sync_info=mybir.SyncInfo(on_wait=chunk, on_update=[]),
                        )
                        ctr += 1
                        new_insts.append(nop)
                    s.on_wait = w[n_extra:]
                    changed = True
                new_insts.append(inst)
            if changed:
                b.instructions = new_insts
    return ctr


def build_nc(t_steps=T):
    """Emit the per-core Bass program. t_steps<T builds a truncated model
    (debug only)."""
    nblk = t_steps // BLK
    nt = t_steps * BPC
    gath_tiles = nt // 128

    nc = bass.Bass()
    # ---- DRAM I/O (per core) ----
    tok_d = nc.dram_tensor("tokens", [128, gath_tiles], I32, kind="ExternalInput")
    emb_d = nc.dram_tensor("emb", [V, EP], F16, kind="ExternalInput")
    w1x_d = nc.dram_tensor("w1x", [128, KE, K1, 128], F16, kind="ExternalInput")
    w1h_d = nc.dram_tensor("w1h", [128, K1, K1, 128], F16, kind="ExternalInput")
    b1_d = nc.dram_tensor("b1", [128, K1], F32, kind="ExternalInput")
    w2x_d = nc.dram_tensor("w2x", [128, K1, K2, 128], F16, kind="ExternalInput")
    b2_d = nc.dram_tensor("b2", [128, K2], F32, kind="ExternalInput")
    w2h_d = nc.dram_tensor("w2h", [128, K2, K2, 128], F16, kind="ExternalInput")
    wd1_d = nc.dram_tensor("wd1", [128, K2, D1], F16, kind="ExternalInput")
    bd1_d = nc.dram_tensor("bd1", [D1, 1], F32, kind="ExternalInput")
    wd2_d = nc.dram_tensor("wd2", [D1, D2], F16, kind="ExternalInput")
    bd2_d = nc.dram_tensor("bd2", [D2, 1], F32, kind="ExternalInput")
    wc_d = nc.dram_tensor("wc", [D2, C], F16, kind="ExternalInput")
    bc_d = nc.dram_tensor("bc", [C, 1], F32, kind="ExternalInput")
    ident_d = nc.dram_tensor("ident", [128, 128], F16, kind="ExternalInput")
    out_d = nc.dram_tensor("out", [C, BPC], F32, kind="ExternalOutput")

    with tile.TileContext(nc) as tc:
        with (
            tc.tile_pool(name="const", bufs=1) as cpool,
            tc.tile_pool(name="act", bufs=1) as apool,
            tc.tile_pool(name="gath", bufs=4) as gpool,
            tc.tile_pool(name="xt", bufs=3) as xtpool,
            tc.tile_pool(name="tmp", bufs=8) as tpool,
            tc.tile_pool(name="psb", bufs=2, space="PSUM") as psb,
            tc.tile_pool(name="ps1", bufs=2, space="PSUM") as ps1,
            tc.tile_pool(name="ps2", bufs=2, space="PSUM") as ps2,
        ):
            # ---- load constants (weights/biases/tokens) ----
            def load(dram, shape, dtype):
                t = cpool.tile(shape, dtype, tag=dram.name)
                nc.sync.dma_start(out=t[:], in_=dram[:])
                return t

            tok_sb = load(tok_d, [128, gath_tiles], I32)
            w1x_sb = load(w1x_d, [128, KE, K1, 128], F16)
            w1h_sb = load(w1h_d, [128, K1, K1, 128], F16)
            b1_sb = load(b1_d, [128, K1], F32)
            w2x_sb = load(w2x_d, [128, K1, K2, 128], F16)
            b2_sb = load(b2_d, [128, K2], F32)
            w2h_sb = load(w2h_d, [128, K2, K2, 128], F16)
            wd1_sb = load(wd1_d, [128, K2, D1], F16)
            bd1_sb = load(bd1_d, [D1, 1], F32)
            wd2_sb = load(wd2_d, [D1, D2], F16)
            bd2_sb = load(bd2_d, [D2, 1], F32)
            wc_sb = load(wc_d, [D2, C], F16)
            bc_sb = load(bc_d, [C, 1], F32)
            ident_sb = load(ident_d, [128, 128], F16)

            # ---- persistent activation buffers (transposed layouts) ----
            # xw1T / xw2T: [feat_chunk, chunk, (t,b)] fp16
            xw1t = apool.tile([128, K1, nt], F16, tag="xw1t")
            xw2t = apool.tile([128, K2, nt], F16, tag="xw2t")
            # seq1T doubles as RNN1 state history; col 0:8 is h0=0,
            # step t writes cols 8+8t : 16+8t.
            seq1t = apool.tile([128, K1, nt + BPC], F16, tag="seq1t")
            # RNN2 state ping-pong: cols 0:8 zeros, slots at 8:16, 16:24.
            h2t = apool.tile([128, K2, 3 * BPC], F16, tag="h2t")
            out_sb = apool.tile([C, BPC], F32, tag="out_sb")

            nc.vector.memzero(seq1t[:, :, 0:BPC])
            nc.vector.memzero(h2t[:])

            # ---- embedding gather + transpose, per 128-token tile ----
            # xT block tiles [128, KE, 512] fp16 feed the xw1 bulk matmul.
            def bulk_proj_cols(w_sb, n_k, n_m, rhs_sl, bias_sb, dst, dst_col,
                               ncol):
                """dst[:, m, dst_col:+ncol] = W.T @ rhs + bias."""
                for m in range(n_m):
                    ps = psb.tile([128, ncol], F32, tag="psb")
                    for k in range(n_k):
                        nc.tensor.matmul(
                            out=ps[:],
                            lhsT=w_sb[:, k, m, :],
                            rhs=rhs_sl[:, k, :],
                            start=(k == 0),
                            stop=(k == n_k - 1),
                        )
                    nc.scalar.activation(
                        out=dst[:, m, dst_col : dst_col + ncol],
                        in_=ps[:],
                        func=AF.Identity,
                        bias=bias_sb[:, m : m + 1],
                        scale=1.0,
                    )

            xt_tiles = []
            gpb = NCOL_BLK // 128  # gather tiles per block
            tr_engines = [nc.sync, nc.scalar]
            for blk in range(nblk):
                xt = xtpool.tile([128, KE, NCOL_BLK], F16, tag="xt")
                xt_tiles.append(xt)
                for gi in range(gpb):
                    g = blk * gpb + gi
                    gt = gpool.tile([128, EP], F16, tag="gt")
                    nc.gpsimd.indirect_dma_start(
                        out=gt[:],
                        out_offset=None,
                        in_=emb_d[:],
                        in_offset=bass.IndirectOffsetOnAxis(
                            ap=tok_sb[:, g : g + 1], axis=0
                        ),
                    )
                    for c in range(KE):
                        tr_engines[(g * KE + c) % 2].dma_start(
                            out=xt[:, c, gi * 128 : (gi + 1) * 128],
                            in_=gt[:, c * 128 : (c + 1) * 128],
                            transpose=True,
                        )
                    if blk == 0:
                        # make block-0 xw1 available per gather tile so the
                        # rnn pipeline starts ~8us earlier
                        bulk_proj_cols(
                            w1x_sb, KE, K1, xt[:, :, gi * 128 : (gi + 1) * 128],
                            b1_sb, xw1t, gi * 128, 128,
                        )

            # ---- helpers ----
            def bulk_proj(w_sb, n_k, n_m, rhs_sl, bias_sb, dst, dst_col):
                bulk_proj_cols(w_sb, n_k, n_m, rhs_sl, bias_sb, dst, dst_col,
                               NCOL_BLK)

            def rnn_step(w_sb, n_k, xwt, state_sl, dst_sl, pspool, halves,
                         relu_prio=None):
                """dst = relu(xw_t + Wh.T @ state). xw_t enters PSUM via one
                identity matmul per half (covering all its regions), then the
                Wh chunks accumulate; the relu is split into halves on separate
                PSUM banks so the first half overlaps the second's matmuls."""
                for m_lo, m_hi, pstag in halves:
                    nh = m_hi - m_lo
                    ps = pspool.tile([128, nh, BPC], F32, tag=pstag)
                    nc.tensor.matmul(
                        out=ps[:],
                        lhsT=ident_sb[:],
                        rhs=xwt[:, m_lo:m_hi, :],
                        start=True,
                        stop=False,
                        skip_group_check=True,
                    )
                    # k-interleaved across the half's m-regions so matmuls
                    # consuming the freshest state chunks come as late as
                    # possible (hides the previous step's relu latency)
                    for k in range(n_k):
                        for m in range(m_lo, m_hi):
                            nc.tensor.matmul(
                                out=ps[:, m - m_lo, :],
                                lhsT=w_sb[:, k, m, :],
                                rhs=state_sl[:, k, :],
                                start=False,
                                stop=(k == n_k - 1),
                                skip_group_check=True,
                            )
                    if relu_prio is not None:
                        with tc.high_priority(relu_prio):
                            nc.vector.tensor_relu(dst_sl[:, m_lo:m_hi, :], ps[:])
                    else:
                        nc.vector.tensor_relu(dst_sl[:, m_lo:m_hi, :], ps[:])

            def rnn1_step(t):
                rnn_step(
                    w1h_sb, K1,
                    xw1t[:, :, t * BPC : (t + 1) * BPC],
                    seq1t[:, :, t * BPC : (t + 1) * BPC],
                    seq1t[:, :, (t + 1) * BPC : (t + 2) * BPC],
                    ps1, [(0, K1, "r1")],
                )

            def rnn2_step(t):
                src = 0 if t == 0 else (BPC + ((t - 1) % 2) * BPC)
                dst = BPC + (t % 2) * BPC
                rnn_step(
                    w2h_sb, K2,
                    xw2t[:, :, t * BPC : (t + 1) * BPC],
                    h2t[:, :, src : src + BPC],
                    h2t[:, :, dst : dst + BPC],
                    ps2, [(0, 2, "r2a"), (2, 4, "r2b")],
                    relu_prio=40,
                )

            # ---- main pipeline: layer-2 runs SKEW steps behind layer-1 ----
            # SKEW > BLK so block-boundary bulk work (xw2 matmuls + bias
            # copies) never gates the immediately following rnn2 steps.
            SKEW = BLK + 4

            def bulk_xw1(b):
                bulk_proj(w1x_sb, KE, K1, xt_tiles[b][:], b1_sb, xw1t,
                          b * NCOL_BLK)

            def bulk_xw2(b):
                bulk_proj(
                    w2x_sb, K1, K2,
                    seq1t[:, :, BPC + b * NCOL_BLK : BPC + (b + 1) * NCOL_BLK],
                    b2_sb, xw2t, b * NCOL_BLK,
                )

            # (block-0 xw1 was emitted per gather tile above)
            for blk in range(nblk):
                for ti in range(BLK):
                    if ti == BLK // 2 and blk + 1 < nblk:
                        bulk_xw1(blk + 1)
                    # rnn2 first: its relus are the critical chain and must
                    # not queue behind rnn1's relu on the in-order DVE
                    t2 = blk * BLK + ti - SKEW
                    if t2 >= 0:
                        rnn2_step(t2)
                    rnn1_step(blk * BLK + ti)
                bulk_xw2(blk)
            for t2 in range(max(0, nblk * BLK - SKEW), t_steps):
                rnn2_step(t2)

            # ---- dense head on the final RNN2 state ----
            t_last = t_steps - 1
            hfin = h2t[:, :, BPC + (t_last % 2) * BPC : 2 * BPC + (t_last % 2) * BPC]

            ps = ps1.tile([D1, BPC], F32, tag="r1")
            for k in range(K2):
                nc.tensor.matmul(out=ps[:], lhsT=wd1_sb[:, k, :], rhs=hfin[:, k, :],
                                 start=(k == 0), stop=(k == K2 - 1))
            d1 = tpool.tile([D1, BPC], F16, tag="d1")
            nc.scalar.activation(out=d1[:], in_=ps[:], func=AF.Relu,
                                 bias=bd1_sb[:, 0:1], scale=1.0)

            ps = ps1.tile([D2, BPC], F32, tag="r1")
            nc.tensor.matmul(out=ps[:], lhsT=wd2_sb[:], rhs=d1[:], start=True,
                             stop=True)
            d2 = tpool.tile([D2, BPC], F16, tag="d2")
            nc.scalar.activation(out=d2[:], in_=ps[:], func=AF.Relu,
                                 bias=bd2_sb[:, 0:1], scale=1.0)

            ps = ps1.tile([C, BPC], F32, tag="r1")
            nc.tensor.matmul(out=ps[:], lhsT=wc_sb[:], rhs=d2[:], start=True,
                             stop=True)
            nc.scalar.activation(out=out_sb[:], in_=ps[:], func=AF.Sigmoid,
                                 bias=bc_sb[:, 0:1], scale=1.0)
            nc.sync.dma_start(out=out_d[:], in_=out_sb[:])

    n_split = _split_excess_waits(nc)
    print(f"[kernel] split {n_split} excess-wait NoOps")
    return nc


# ---------------------------------------------------------------------------
# Host-side input prep


def _chunk_lhsT(w, kc, mc):
    """[K, M] -> [128, kc, mc, 128] fp16 lhsT chunk layout."""
    K, M = w.shape
    assert K == kc * 128 and M == mc * 128
    return np.ascontiguousarray(
        w.reshape(kc, 128, mc, 128).transpose(1, 0, 2, 3)
    ).astype(np.float16)


def prep_core_inputs(inputs, t_steps=T):
    """Returns (shared_weight_map, per_core_token_list)."""
    emb = np.asarray(inputs["emb"], np.float32)
    emb_p = np.zeros((V, EP), np.float16)
    emb_p[:, :E] = emb.astype(np.float16)

    w1x = np.zeros((EP, H1), np.float32)
    w1x[:E] = np.asarray(inputs["W1x"], np.float32)

    shared = {
        "emb": emb_p,
        "w1x": _chunk_lhsT(w1x, KE, K1),
        "w1h": _chunk_lhsT(np.asarray(inputs["W1h"], np.float32), K1, K1),
        "b1": np.ascontiguousarray(
            np.asarray(inputs["b1"], np.float32).reshape(K1, 128).T
        ),
        "w2x": _chunk_lhsT(np.asarray(inputs["W2x"], np.float32), K1, K2),
        "b2": np.ascontiguousarray(
            np.asarray(inputs["b2"], np.float32).reshape(K2, 128).T
        ),
        "w2h": _chunk_lhsT(np.asarray(inputs["W2h"], np.float32), K2, K2),
        "wd1": np.ascontiguousarray(
            np.asarray(inputs["Wd1"], np.float32).reshape(K2, 128, D1)
            .transpose(1, 0, 2)
        ).astype(np.float16),
        "bd1": np.asarray(inputs["bd1"], np.float32).reshape(D1, 1),
        "wd2": np.asarray(inputs["Wd2"], np.float32).astype(np.float16),
        "bd2": np.asarray(inputs["bd2"], np.float32).reshape(D2, 1),
        "wc": np.asarray(inputs["Wc"], np.float32).astype(np.float16),
        "bc": np.asarray(inputs["bc"], np.float32).reshape(C, 1),
        "ident": np.eye(128, dtype=np.float16),
    }

    tokens = np.asarray(inputs["tokens"], np.int32)
    per_core_tok = []
    gath_tiles = (t_steps * BPC) // 128
    for c in range(N_CORES):
        cols = tokens[c * BPC : (c + 1) * BPC, :t_steps].T.reshape(-1)  # (t,b)
        per_core_tok.append(
            np.ascontiguousarray(cols.reshape(gath_tiles, 128).T)
        )
    return shared, per_core_tok


_CACHE = {}


def run(inputs, t_steps=T, trace=False):
    key = t_steps
    if key not in _CACHE:
        _CACHE[key] = build_nc(t_steps)
    nc = _CACHE[key]
    shared, per_core_tok = prep_core_inputs(inputs, t_steps)
    in_maps = [dict(shared, tokens=per_core_tok[c]) for c in range(N_CORES)]
    res = run_bass_kernel_spmd(
        nc, in_maps, core_ids=list(range(N_CORES)), trace=trace
    )
    out = np.concatenate(
        [res.results[c]["out"].reshape(BPC, C) for c in range(N_CORES)], axis=0
    )
    return out.astype(np.float32), res


def kernel(**inputs):
    out, _ = run(inputs)
    return out



# revision 15
# speedup vs baseline: 1.0096x; 1.0096x over previous
"""Trainium2 Bass kernel for a 2-layer SimpleRNN classifier.

Model (per reference):
  x = emb[tokens]                               # [B,T,E]
  seq1 = SimpleRNN_relu(x;  W1x, W1h, b1)       # [B,T,H1], return_sequences
  h    = SimpleRNN_relu(seq1; W2x, W2h, b2)[-1] # [B,H2], last step
  h = relu(h@Wd1+bd1); h = relu(h@Wd2+bd2); out = sigmoid(h@Wc+bc)  # [B,1]

Sharding: data-parallel over batch, 8 rows per core on 8 NeuronCores.

Device design notes (v2):
  - All activations transposed on-chip: features on partitions, (t,b) cols.
  - Input projections (x@W1x, seq1@W2x) are bulk matmuls that write
    DIRECTLY into the PSUM banks the recurrent matmuls then accumulate
    onto (start=False) -- no identity-injection matmuls, no SBUF xw
    buffers, no bias copies.
  - b1 rides in a constant-1.0 padding column of the embedding (W1x pad
    row holds b1); b2 rides on contraction-1 matmuls per block.
  - RNN state in small dedicated SBUF tiles (lo/hi per ping-pong slot) so
    relu deps are exact (no false tile-granular WARs).
  - Embedding gather tiles are transposed on the PE (is_transpose matmul
    via PSUM) instead of slow transposing DMAs.
  - rnn2 relus on DVE (split lo/hi for pipelining), rnn1 relu on the Act
    engine; PE stream is one LDWEIGHTS+MATMUL pair per 128x128 weight
    chunk per step, which is the weight-load-bound roofline.
  - Optional fp8(e4m3) weight storage (WDT/SCALE): halves LDWEIGHTS time
    via FWL; PSUM holds SCALE*z, relus rescale by 1/SCALE.
"""

import numpy as np

import concourse.bass as bass
import concourse.mybir as mybir
import concourse.tile as tile
from concourse.bass_utils import run_bass_kernel_spmd

# ---------------------------------------------------------------------------
# Problem constants (hardcoded per the task contract).
B, T, V, E = 64, 512, 50000, 300
H1, H2, D1, D2, C = 256, 512, 128, 64, 1
N_CORES = 8
BPC = B // N_CORES          # batch rows per core = 8
NT = T * BPC                # columns of the transposed activation = 4096
EP = 384                    # E padded to 3 partition chunks (col E holds 1.0)
KE, K1, K2 = EP // 128, H1 // 128, H2 // 128   # 3, 2, 4
BLK = 32                    # time steps per block (= PSUM bank of 32*8 cols)
NBLK = T // BLK             # 16
NCOL = BLK * BPC            # 256 activation columns per block
GPB = NCOL // 128           # gather tiles per block = 2

F16 = mybir.dt.float16
F32 = mybir.dt.float32
I32 = mybir.dt.int32
AF = mybir.ActivationFunctionType
ALU = mybir.AluOpType

# Weight storage dtype for the four projection/recurrent matrices.
# fp16: LDWEIGHTS 26.7ns/tile. fp8e4: 13.3ns/tile (FWL quad rate).
WDT = F16
SCALE = 1.0                 # weights stored as SCALE*W; relus rescale by 1/SCALE

MAX_WAITS = 1  # walrus in this container rejects more sem waits per inst
DEBUG_DUMP = False  # add DRAM dumps of xt0/seq1t/h2 for bisection


def _split_excess_waits(nc, max_waits=MAX_WAITS):
    """The container's walrus codegen rejects instructions carrying more than
    a couple of sem waits ("Too many sync wait commands"). Tile freely attaches
    many. Post-process the scheduled BIR: move excess waits onto injected NoOps
    placed immediately before the instruction on the same engine (engines
    process waits in instruction order, so semantics are preserved)."""
    ctr = 0
    for f in nc.m.functions:
        for b in f.blocks:
            new_insts = []
            changed = False
            for inst in b.instructions:
                s = inst.sync_info
                if s is not None and s.on_wait and len(s.on_wait) > max_waits:
                    w = list(s.on_wait)
                    n_extra = len(w) - max_waits
                    for i in range(0, n_extra, max_waits):
                        chunk = w[i : min(i + max_waits, n_extra)]
                        nop = mybir.InstNoOp(
                            name=f"bass_waitsplit_{ctr}",
                            engine=inst.engine,
                            ins=[],
                            outs=[],
                            sync_info=mybir.SyncInfo(on_wait=chunk, on_update=[]),
                        )
                        ctr += 1
                        new_insts.append(nop)
                    s.on_wait = w[n_extra:]
                    changed = True
                new_insts.append(inst)
            if changed:
                b.instructions = new_insts
    return ctr


def build_nc(t_steps=T):
    """Emit the per-core Bass program. t_steps<T builds a truncated model
    (debug only; t_steps must be a multiple of BLK)."""
    assert t_steps % BLK == 0
    nblk = t_steps // BLK
    nt = t_steps * BPC
    n_gath = nt // 128

    nc = bass.Bass()
    # ---- DRAM I/O (per core) ----
    tok_d = nc.dram_tensor("tokens", [128, n_gath], I32, kind="ExternalInput")
    emb_d = nc.dram_tensor("emb", [V, EP], F16, kind="ExternalInput")
    w1x_d = nc.dram_tensor("w1x", [128, KE, K1, 128], WDT, kind="ExternalInput")
    w1h_d = nc.dram_tensor("w1h", [128, K1, K1, 128], WDT, kind="ExternalInput")
    w2x_d = nc.dram_tensor("w2x", [128, K1, K2, 128], WDT, kind="ExternalInput")
    w2h_d = nc.dram_tensor("w2h", [128, K2, K2, 128], WDT, kind="ExternalInput")
    b2r_d = nc.dram_tensor("b2r", [1, K2, 128], F16, kind="ExternalInput")
    wd1_d = nc.dram_tensor("wd1", [128, K2, D1], F16, kind="ExternalInput")
    bd1_d = nc.dram_tensor("bd1", [D1, 1], F32, kind="ExternalInput")
    wd2_d = nc.dram_tensor("wd2", [D1, D2], F16, kind="ExternalInput")
    bd2_d = nc.dram_tensor("bd2", [D2, 1], F32, kind="ExternalInput")
    wc_d = nc.dram_tensor("wc", [D2, C], F16, kind="ExternalInput")
    bc_d = nc.dram_tensor("bc", [C, 1], F32, kind="ExternalInput")
    ident_d = nc.dram_tensor("ident", [128, 128], F16, kind="ExternalInput")
    out_d = nc.dram_tensor("out", [C, BPC], F32, kind="ExternalOutput")
    if DEBUG_DUMP:
        xt0_d = nc.dram_tensor("dbg_xt0", [128, KE, NCOL], F16,
                               kind="ExternalOutput")
        seq_d = nc.dram_tensor("dbg_seq", [128, K1, nt + BPC], F16,
                               kind="ExternalOutput")
        h2_d = nc.dram_tensor("dbg_h2", [128, K2, BPC], F16,
                              kind="ExternalOutput")
        ps1_d = nc.dram_tensor("dbg_ps1", [128, K1, NCOL], F32,
                               kind="ExternalOutput")

    inv_s = 1.0 / SCALE

    with tile.TileContext(nc) as tc:
        with (
            tc.tile_pool(name="const", bufs=1) as cpool,
            tc.tile_pool(name="act", bufs=1) as apool,
            tc.tile_pool(name="gath", bufs=4) as gpool,
            tc.tile_pool(name="xt", bufs=3) as xtpool,
            tc.tile_pool(name="head", bufs=4) as hpool,
            tc.tile_pool(name="ps", bufs=1, space="PSUM") as pspool,
        ):
            # ---- load constants; order matters for startup latency ----
            def load(dram, shape, dtype):
                t = cpool.tile(shape, dtype, tag=dram.name)
                nc.sync.dma_start(out=t[:], in_=dram[:])
                return t

            tok_sb = load(tok_d, [128, n_gath], I32)
            ident_sb = load(ident_d, [128, 128], F16)
            w1x_sb = load(w1x_d, [128, KE, K1, 128], WDT)
            w1h_sb = load(w1h_d, [128, K1, K1, 128], WDT)
            w2x_sb = load(w2x_d, [128, K1, K2, 128], WDT)
            w2h_sb = load(w2h_d, [128, K2, K2, 128], WDT)
            b2r_sb = load(b2r_d, [1, K2, 128], F16)
            wd1_sb = load(wd1_d, [128, K2, D1], F16)
            bd1_sb = load(bd1_d, [D1, 1], F32)
            wd2_sb = load(wd2_d, [D1, D2], F16)
            bd2_sb = load(bd2_d, [D2, 1], F32)
            wc_sb = load(wc_d, [D2, C], F16)
            bc_sb = load(bc_d, [C, 1], F32)

            # ---- persistent SBUF state ----
            seq1t = apool.tile([128, K1, nt + BPC], F16, tag="seq1t")
            h2z = apool.tile([128, K2, BPC], F16, tag="h2z")
            h2_lo = [
                apool.tile([128, 2, BPC], F16, tag=f"h2lo{s}", name=f"h2lo{s}")
                for s in (0, 1)
            ]
            h2_hi = [
                apool.tile([128, 2, BPC], F16, tag=f"h2hi{s}", name=f"h2hi{s}")
                for s in (0, 1)
            ]
            ones_sb = apool.tile([1, NCOL], F16, tag="ones")
            out_sb = apool.tile([C, BPC], F32, tag="out_sb")

            nc.vector.memzero(seq1t[:, :, 0:BPC])
            nc.vector.memzero(h2z[:])
            nc.vector.memset(ones_sb[0:1, :], 1.0)

            # ---- PSUM banks (8 x 2KB exactly) ----
            pr1 = [pspool.tile([128, K1, NCOL], F32, tag=f"pr1_{i}",
                               name=f"pr1_{i}") for i in (0, 1)]
            pr2lo = [pspool.tile([128, 2, NCOL], F32, tag=f"pr2lo_{i}",
                                 name=f"pr2lo_{i}") for i in (0, 1)]
            pr2hi = [pspool.tile([128, 2, NCOL], F32, tag=f"pr2hi_{i}",
                                 name=f"pr2hi_{i}") for i in (0, 1)]
            # transpose scratch; the dense head reuses this tag at the end
            ptr_tag = "ptr"

            def ptile(shape, dtype):
                return pspool.tile(shape, dtype, tag=ptr_tag, bufs=2,
                                   name="ptr")

            # ---- embedding gather -> PE transpose -> xt (feature-major) ----
            xt_tiles = [None] * nblk

            def gather_tile(b, gi):
                g = b * GPB + gi
                if xt_tiles[b] is None:
                    xt_tiles[b] = xtpool.tile([128, KE, NCOL], F16, tag="xt",
                                              name="xt")
                xt = xt_tiles[b]
                gt = gpool.tile([128, EP], F16, tag="gt", name="gt")
                nc.gpsimd.indirect_dma_start(
                    out=gt[:],
                    out_offset=None,
                    in_=emb_d[:],
                    in_offset=bass.IndirectOffsetOnAxis(
                        ap=tok_sb[:, g : g + 1], axis=0
                    ),
                )
                pt = ptile([128, KE, 128], F16)
                for c in range(KE):
                    nc.tensor.transpose(
                        out=pt[:, c, :], in_=gt[:, c * 128 : (c + 1) * 128],
                        identity=ident_sb[:],
                    )
                nc.scalar.copy(
                    out=xt[:, :, gi * 128 : (gi + 1) * 128], in_=pt[:]
                )

            # ---- bulk input projections straight into PSUM ----
            def bulk_xw1(b, gi=None):
                # NOTE: start=True clears the has_written bits of the WHOLE
                # bank, so it must be emitted exactly once per bank per block
                # (first touch); later writes use start=False, which
                # write-throughs where has_written is clear and adds where set.
                bank = pr1[b % 2]
                lo, hi = (0, NCOL) if gi is None else (gi * 128, (gi + 1) * 128)
                for m in range(K1):
                    for k in range(KE):
                        nc.tensor.matmul(
                            out=bank[:, m, lo:hi],
                            lhsT=w1x_sb[:, k, m, :],
                            rhs=xt_tiles[b][:, k, lo:hi],
                            start=(k == 0 and m == 0 and gi in (None, 0)),
                            stop=False,
                            skip_group_check=True,
                        )

            def bulk_xw2(b):
                cols = seq1t[:, :, BPC + b * NCOL : BPC + (b + 1) * NCOL]
                for half, bank in ((0, pr2lo[b % 2]), (1, pr2hi[b % 2])):
                    for ml in range(2):
                        m = half * 2 + ml
                        for k in range(K1):
                            nc.tensor.matmul(
                                out=bank[:, ml, :],
                                lhsT=w2x_sb[:, k, m, :],
                                rhs=cols[:, k, :],
                                start=(k == 0 and ml == 0),
                                stop=False,
                                skip_group_check=True,
                            )
                        nc.tensor.matmul(
                            out=bank[:, ml, :],
                            lhsT=b2r_sb[0:1, m, :],
                            rhs=ones_sb[0:1, :],
                            start=False,
                            stop=False,
                            skip_group_check=True,
                        )

            # ---- per-step recurrences ----
            def relu_dve(out, in0, prio=None):
                def emit():
                    if SCALE == 1.0:
                        nc.vector.tensor_scalar(out, in0, 0.0, None, ALU.max)
                    else:
                        nc.vector.tensor_scalar(
                            out, in0, inv_s, 0.0, ALU.mult, ALU.max
                        )
                if prio is not None:
                    with tc.high_priority(prio):
                        emit()
                else:
                    emit()

            def h2_src(t2):
                if t2 == 0:
                    return [h2z[:, k, :] for k in range(K2)]
                lo, hi = h2_lo[(t2 - 1) % 2], h2_hi[(t2 - 1) % 2]
                return [lo[:, 0, :], lo[:, 1, :], hi[:, 0, :], hi[:, 1, :]]

            def rnn2_step(t2):
                bank_lo = pr2lo[(t2 // BLK) % 2]
                bank_hi = pr2hi[(t2 // BLK) % 2]
                s = t2 % BLK
                src = h2_src(t2)
                for half, bank, dst in (
                    (0, bank_lo, h2_lo[t2 % 2]),
                    (1, bank_hi, h2_hi[t2 % 2]),
                ):
                    for k in range(K2):
                        for ml in range(2):
                            nc.tensor.matmul(
                                out=bank[:, ml, s * BPC : (s + 1) * BPC],
                                lhsT=w2h_sb[:, k, half * 2 + ml, :],
                                rhs=src[k],
                                start=False,
                                stop=(k == K2 - 1),
                                skip_group_check=True,
                            )
                    relu_dve(
                        dst[:], bank[:, :, s * BPC : (s + 1) * BPC], prio=40
                    )

            def rnn1_step(t):
                bank = pr1[(t // BLK) % 2]
                s = t % BLK
                for k in range(K1):
                    src = seq1t[:, k, t * BPC : (t + 1) * BPC]
                    for m in range(K1):
                        nc.tensor.matmul(
                            out=bank[:, m, s * BPC : (s + 1) * BPC],
                            lhsT=w1h_sb[:, k, m, :],
                            rhs=src,
                            start=False,
                            stop=(k == K1 - 1),
                            skip_group_check=True,
                        )
                relu_dve(
                    seq1t[:, :, (t + 1) * BPC : (t + 2) * BPC],
                    bank[:, :, s * BPC : (s + 1) * BPC],
                )

            # ---- prologue: blocks 0+1 gathers; block-0 xw1 per gather tile
            for gi in range(GPB):
                gather_tile(0, gi)
                bulk_xw1(0, gi)
            for gi in range(GPB):
                gather_tile(1, gi)

            # ---- main pipeline: rnn2 runs SKEW steps behind rnn1 ----
            SKEW = BLK + 4
            GATHER_TIS = {6: 0, 22: 1}  # ti -> gather tile gi of block b+2

            for blk in range(nblk):
                for ti in range(BLK):
                    t = blk * BLK + ti
                    if ti in GATHER_TIS and blk + 2 < nblk:
                        gather_tile(blk + 2, GATHER_TIS[ti])
                    if ti == BLK // 2 and blk + 1 < nblk:
                        bulk_xw1(blk + 1)
                    t2 = t - SKEW
                    if t2 >= 0:
                        rnn2_step(t2)
                    rnn1_step(t)
                bulk_xw2(blk)
            for t2 in range(max(0, t_steps - SKEW), t_steps):
                rnn2_step(t2)

            # ---- dense head on the final RNN2 state ----
            t_last = t_steps - 1
            hfin = h2_src(t_last + 1)

            if DEBUG_DUMP:
                nc.sync.dma_start(out=xt0_d[:], in_=xt_tiles[0][:])
                nc.sync.dma_start(out=seq_d[:], in_=seq1t[:])
                lo, hi = h2_lo[t_last % 2], h2_hi[t_last % 2]
                nc.sync.dma_start(out=h2_d[:, 0:2, :], in_=lo[:])
                nc.sync.dma_start(out=h2_d[:, 2:4, :], in_=hi[:])
                ps1cp = apool.tile([128, K1, NCOL], F32, tag="dbg_ps1cp",
                                   name="ps1cp")
                nc.vector.tensor_scalar(ps1cp[:], pr1[0][:], 1.0, None,
                                        ALU.mult)
                nc.sync.dma_start(out=ps1_d[:], in_=ps1cp[:])

            ps = ptile([D1, BPC], F32)
            for k in range(K2):
                nc.tensor.matmul(out=ps[:], lhsT=wd1_sb[:, k, :], rhs=hfin[k],
                                 start=(k == 0), stop=(k == K2 - 1),
                                 skip_group_check=True)
            d1 = hpool.tile([D1, BPC], F16, tag="d1")
            nc.scalar.activation(out=d1[:], in_=ps[:], func=AF.Relu,
                                 bias=bd1_sb[:, 0:1], scale=1.0)

            ps = ptile([D2, BPC], F32)
            nc.tensor.matmul(out=ps[:], lhsT=wd2_sb[:], rhs=d1[:], start=True,
                             stop=True, skip_group_check=True)
            d2 = hpool.tile([D2, BPC], F16, tag="d2")
            nc.scalar.activation(out=d2[:], in_=ps[:], func=AF.Relu,
                                 bias=bd2_sb[:, 0:1], scale=1.0)

            ps = ptile([C, BPC], F32)
            nc.tensor.matmul(out=ps[:], lhsT=wc_sb[:], rhs=d2[:], start=True,
                             stop=True, skip_group_check=True)
            nc.scalar.activation(out=out_sb[:], in_=ps[:], func=AF.Sigmoid,
                                 bias=bc_sb[:, 0:1], scale=1.0)
            nc.sync.dma_start(out=out_d[:], in_=out_sb[:])

    n_split = _split_excess_waits(nc)
    print(f"[kernel] split {n_split} excess-wait NoOps")
    return nc


# ---------------------------------------------------------------------------
# Host-side input prep


def _to_wdt(w):
    """Scale and store in the weight dtype."""
    w = np.asarray(w, np.float32) * SCALE
    return w.astype(mybir.dt.np(WDT))


def _chunk_lhsT(w, kc, mc):
    """[K, M] -> [128, kc, mc, 128] lhsT chunk layout in WDT."""
    K, M = w.shape
    assert K == kc * 128 and M == mc * 128
    return np.ascontiguousarray(
        _to_wdt(w).reshape(kc, 128, mc, 128).transpose(1, 0, 2, 3)
    )


def prep_core_inputs(inputs, t_steps=T):
    """Returns (shared_weight_map, per_core_token_list)."""
    emb = np.asarray(inputs["emb"], np.float32)
    emb_p = np.zeros((V, EP), np.float16)
    emb_p[:, :E] = emb.astype(np.float16)
    emb_p[:, E] = 1.0  # constant column carrying b1 via W1x pad row E

    w1x = np.zeros((EP, H1), np.float32)
    w1x[:E] = np.asarray(inputs["W1x"], np.float32)
    w1x[E] = np.asarray(inputs["b1"], np.float32)

    shared = {
        "emb": emb_p,
        "w1x": _chunk_lhsT(w1x, KE, K1),
        "w1h": _chunk_lhsT(np.asarray(inputs["W1h"], np.float32), K1, K1),
        "w2x": _chunk_lhsT(np.asarray(inputs["W2x"], np.float32), K1, K2),
        "w2h": _chunk_lhsT(np.asarray(inputs["W2h"], np.float32), K2, K2),
        "b2r": np.ascontiguousarray(
            (np.asarray(inputs["b2"], np.float32) * SCALE)
            .reshape(1, K2, 128).astype(np.float16)
        ),
        "wd1": np.ascontiguousarray(
            np.asarray(inputs["Wd1"], np.float32).reshape(K2, 128, D1)
            .transpose(1, 0, 2)
        ).astype(np.float16),
        "bd1": np.asarray(inputs["bd1"], np.float32).reshape(D1, 1),
        "wd2": np.asarray(inputs["Wd2"], np.float32).astype(np.float16),
        "bd2": np.asarray(inputs["bd2"], np.float32).reshape(D2, 1),
        "wc": np.asarray(inputs["Wc"], np.float32).astype(np.float16),
        "bc": np.asarray(inputs["bc"], np.float32).reshape(C, 1),
        "ident": np.eye(128, dtype=np.float16),
    }

    tokens = np.asarray(inputs["tokens"], np.int32)
    per_core_tok = []
    n_gath = (t_steps * BPC) // 128
    for c in range(N_CORES):
        cols = tokens[c * BPC : (c + 1) * BPC, :t_steps].T.reshape(-1)  # (t,b)
        per_core_tok.append(
            np.ascontiguousarray(cols.reshape(n_gath, 128).T)
        )
    return shared, per_core_tok


_CACHE = {}


def run(inputs, t_steps=T, trace=False):
    key = t_steps
    if key not in _CACHE:
        _CACHE[key] = build_nc(t_steps)
    nc = _CACHE[key]
    shared, per_core_tok = prep_core_inputs(inputs, t_steps)
    in_maps = [dict(shared, tokens=per_core_tok[c]) for c in range(N_CORES)]
    res = run_bass_kernel_spmd(
        nc, in_maps, core_ids=list(range(N_CORES)), trace=trace
    )
    out = np.concatenate(
        [res.results[c]["out"].reshape(BPC, C) for c in range(N_CORES)], axis=0
    )
    return out.astype(np.float32), res


def kernel(**inputs):
    out, _ = run(inputs)
    return out
